# revision 28
# baseline (speedup 1.0000x reference)
"""MoE top-2 routing kernel for Trainium2 (8 NeuronCores, expert-parallel).

Host: gating softmax + top-2 (float64 numpy), per-expert token gather,
weight re-layout + bf16 cast. Device (per core, SPMD): one expert's MLP
   h = relu(x @ W1 + b1); y = h @ W2
over that expert's routed tokens, bf16 matmuls with fp32 PSUM accumulation.
Host: combine y * gate (+ b2 folded in) via scatter-add.

Device layout: tokens in the matmul free dim, feature dims in the partition
dim, so the two layers chain without transposes and b1 is a per-partition
activation bias. Output is [D, cap] per core, transposed back on host.

DMA notes (measured): issue costs ~600ns of engine time per dma_start, a
queue ramps from ~60 to ~400GB/s over ~30us, the gpsimd queue is slow
(~40GB/s), and non-sequential DRAM reads halve throughput. So all host
arrays are laid out exactly as their SBUF destination (every DMA is a 2D
contiguous copy), the PE-start critical path (x chunk 0) is split across
the two fast queues (sync + scalar), w1 streams on sync in fine-grained
groups so layer 1 of chunk 0 is never starved, and only the non-critical
later x chunks ride the slow gpsimd queue.
"""

import numpy as np
import ml_dtypes

import concourse.bass as bass
from concourse import mybir
from concourse.bass_utils import run_bass_kernel_spmd

D = 1024
HID = 4096
E = 8
TOP_K = 2
KD = D // 128     # 8  k-blocks for layer 1
MH = HID // 128   # 32 m-blocks for layer 1 / k-blocks for layer 2
MD = D // 128     # 8  m-blocks for layer 2
TCMAX = 512       # max matmul free dim (one fp32 PSUM bank)

BF16 = ml_dtypes.bfloat16

# w1 m-block DMA groups: fine-grained (2 blocks = 512KB each) so layer 1 of
# chunk 0 rides the stream smoothly while the DMA clocks ramp up
W1_GROUPS = [(m, m + 2) for m in range(0, MH, 2)]
W2_GROUPS = [(0, 4), (4, 8)]

# first chunk is small: its layer-1 compute is paced by w1 arrival anyway
# (DMA bandwidth ramps over the first ~40us), so keep it near the ride point
TC0 = 256


def _chunks_of(cap: int):
    """Token chunks: small first chunk (rides the weight stream), remainder
    split evenly into <=512 multiple-of-4 pieces."""
    out = []
    first = min(TC0, cap)
    out.append((0, 0, first))
    t0 = first
    rest = cap - t0
    if rest > 0:
        n = -(-rest // TCMAX)
        base = -(-rest // (4 * n)) * 4
        for _ in range(n):
            tc = min(base, cap - t0)
            out.append((len(out), t0, tc))
            t0 += tc
    return out


def _build_program(cap: int):
    chunks = _chunks_of(cap)

    nc = bass.Bass()

    # all DRAM tensors are laid out exactly as their SBUF destinations
    xTd = nc.dram_tensor("xT", [128, KD * cap], mybir.dt.bfloat16, kind="ExternalInput")
    w1d = nc.dram_tensor("w1", [128, MH * KD * 128], mybir.dt.bfloat16, kind="ExternalInput")
    b1d = nc.dram_tensor("b1t", [128, MH], mybir.dt.float32, kind="ExternalInput")
    w2d = nc.dram_tensor("w2", [128, MD * MH * 128], mybir.dt.bfloat16, kind="ExternalInput")
    outd = nc.dram_tensor("outT", [MD, 128, cap], mybir.dt.float32, kind="ExternalOutput")

    from contextlib import ExitStack

    with ExitStack() as ctx:
        w1_sb = ctx.enter_context(nc.sbuf_tensor("w1_sb", [128, MH * KD * 128], mybir.dt.bfloat16))
        w2_sb = ctx.enter_context(nc.sbuf_tensor("w2_sb", [128, MD * MH * 128], mybir.dt.bfloat16))
        x_sb = ctx.enter_context(nc.sbuf_tensor("x_sb", [128, KD * cap], mybir.dt.bfloat16))
        h_sb = ctx.enter_context(nc.sbuf_tensor("h_sb", [128, MH * TCMAX], mybir.dt.bfloat16))
        o_sb = ctx.enter_context(nc.sbuf_tensor("o_sb", [128, 2 * TCMAX], mybir.dt.float32))
        b1_sb = ctx.enter_context(nc.sbuf_tensor("b1_sb", [128, MH], mybir.dt.float32))
        pt1a = ctx.enter_context(nc.psum_tensor("pt1a", [128, TCMAX], mybir.dt.float32))
        pt1b = ctx.enter_context(nc.psum_tensor("pt1b", [128, TCMAX], mybir.dt.float32))
        pt2a = ctx.enter_context(nc.psum_tensor("pt2a", [128, TCMAX], mybir.dt.float32))
        pt2b = ctx.enter_context(nc.psum_tensor("pt2b", [128, TCMAX], mybir.dt.float32))
        dma_misc = ctx.enter_context(nc.semaphore("dma_misc"))  # b1 load
        dma_xs = ctx.enter_context(nc.semaphore("dma_xs"))      # x chunk-0 half, sync
        dma_xa = ctx.enter_context(nc.semaphore("dma_xa"))      # x chunk-0 half, scalar
        dma_xg = ctx.enter_context(nc.semaphore("dma_xg"))      # +16 per later x chunk, gpsimd
        dma_w1 = ctx.enter_context(nc.semaphore("dma_w1"))      # +16 per w1 group DMA
        dma_w2 = ctx.enter_context(nc.semaphore("dma_w2"))      # +16 per w2 group DMA
        dma_oe = ctx.enter_context(nc.semaphore("dma_oe"))      # +16 per out DMA (sync)
        pe1_sem = ctx.enter_context(nc.semaphore("pe1_sem"))    # +1 per finished L1 m-group
        pe2_sem = ctx.enter_context(nc.semaphore("pe2_sem"))    # +1 per finished L2 mo-group
        act1_sem = ctx.enter_context(nc.semaphore("act1_sem"))  # +1 per L1 psum evict (relu)
        dve_sem = ctx.enter_context(nc.semaphore("dve_sem"))    # +1 per L2 psum evict (copy)
        block = ctx.enter_context(nc.Block())

        pt1 = [pt1a, pt1b]
        pt2 = [pt2a, pt2b]
        # chunk-major x column offsets (shared by DRAM and SBUF layouts)
        xoff = []
        off = 0
        for c, t0, tc in chunks:
            xoff.append(off)
            off += KD * tc
        tc0 = chunks[0][2]
        xhalf = KD // 2 * tc0  # split point of chunk 0's columns

        # w1 wait threshold (on dma_w1) per m-block; emitted where it rises
        w1_thresh = [0] * MH
        for i, (m0, m1) in enumerate(W1_GROUPS):
            for m in range(m0, m1):
                w1_thresh[m] = 16 * (i + 1)
        w2_thresh = [0] * MD
        for i, (m0, m1) in enumerate(W2_GROUPS):
            for m in range(m0, m1):
                w2_thresh[m] = 16 * (i + 1)

        @block.sync
        def _(sync):
            sync.dma_start(
                out=x_sb[:, 0: xhalf], in_=xTd[:, 0: xhalf],
            ).then_inc(dma_xs, 16)
            for i, (m0, m1) in enumerate(W1_GROUPS):
                sync.dma_start(
                    out=w1_sb[:, m0 * KD * 128: m1 * KD * 128],
                    in_=w1d[:, m0 * KD * 128: m1 * KD * 128],
                ).then_inc(dma_w1, 16)
            for c, t0, tc in chunks:
                for mo in range(MD):
                    g = c * MD + mo
                    sync.wait_ge(dve_sem, g + 1)
                    sync.dma_start(
                        out=outd[mo, :, t0: t0 + tc],
                        in_=o_sb[:, (g % 2) * TCMAX: (g % 2) * TCMAX + tc],
                    ).then_inc(dma_oe, 16)

        @block.scalar
        def _(scalar):
            scalar.dma_start(
                out=x_sb[:, xhalf: KD * tc0], in_=xTd[:, xhalf: KD * tc0],
            ).then_inc(dma_xa, 16)
            scalar.dma_start(out=b1_sb[:], in_=b1d[:]).then_inc(dma_misc, 16)
            for i, (m0, m1) in enumerate(W2_GROUPS):
                scalar.dma_start(
                    out=w2_sb[:, m0 * MH * 128: m1 * MH * 128],
                    in_=w2d[:, m0 * MH * 128: m1 * MH * 128],
                ).then_inc(dma_w2, 16)
            scalar.wait_ge(dma_misc, 16)
            for c, t0, tc in chunks:
                for m in range(MH):
                    if m == 0 and c > 0:
                        # h_sb reused: wait until L2 of chunk c-1 consumed it
                        scalar.wait_ge(pe2_sem, MD * c)
                    scalar.wait_ge(pe1_sem, c * MH + m + 1)
                    scalar.activation(
                        h_sb[:, m * TCMAX: m * TCMAX + tc],
                        pt1[m % 2][:, :tc],
                        mybir.ActivationFunctionType.Relu,
                        bias=b1_sb[:, m: m + 1],
                    ).then_inc(act1_sem, 1)

        @block.gpsimd
        def _(gpsimd):
            for c, t0, tc in chunks[1:]:
                gpsimd.dma_start(
                    out=x_sb[:, xoff[c]: xoff[c] + KD * tc],
                    in_=xTd[:, xoff[c]: xoff[c] + KD * tc],
                ).then_inc(dma_xg, 16)

        @block.tensor
        def _(tensor):
            tensor.wait_ge(dma_xs, 16)
            tensor.wait_ge(dma_xa, 16)
            for c, t0, tc in chunks:
                if c > 0:
                    tensor.wait_ge(dma_xg, 16 * c)
                # layer 1: h[m*128+p, t] = relu(sum_d W1[d, m*128+p] x[d, t] + b1)
                for m in range(MH):
                    if c == 0 and (m == 0 or w1_thresh[m] > w1_thresh[m - 1]):
                        tensor.wait_ge(dma_w1, w1_thresh[m])
                    g1 = c * MH + m
                    if g1 >= 2:
                        tensor.wait_ge(act1_sem, g1 - 1)  # psum bank m%2 evicted
                    ps = pt1[m % 2]
                    for k in range(KD):
                        mm = tensor.matmul(
                            ps[:, :tc],
                            w1_sb[:, (m * KD + k) * 128: (m * KD + k + 1) * 128],
                            x_sb[:, xoff[c] + k * tc: xoff[c] + (k + 1) * tc],
                            start=(k == 0),
                            stop=(k == KD - 1),
                        )
                    mm.then_inc(pe1_sem, 1)
                # layer 2: y[mo*128+p, t] = sum_hid W2[hid, mo*128+p] h[hid, t]
                tensor.wait_ge(act1_sem, MH * (c + 1))  # all h of this chunk ready
                for mo in range(MD):
                    if c == 0 and (mo == 0 or w2_thresh[mo] > w2_thresh[mo - 1]):
                        tensor.wait_ge(dma_w2, w2_thresh[mo])
                    g2 = c * MD + mo
                    if g2 >= 2:
                        tensor.wait_ge(dve_sem, g2 - 1)  # psum bank mo%2 evicted
                    ps = pt2[mo % 2]
                    for k in range(MH):
                        mm = tensor.matmul(
                            ps[:, :tc],
                            w2_sb[:, (mo * MH + k) * 128: (mo * MH + k + 1) * 128],
                            h_sb[:, k * TCMAX: k * TCMAX + tc],
                            start=(k == 0),
                            stop=(k == MH - 1),
                        )
                    mm.then_inc(pe2_sem, 1)

        @block.vector
        def _(vector):
            for c, t0, tc in chunks:
                for mo in range(MD):
                    g = c * MD + mo
                    if g >= 2:
                        # o_sb slot g%2 free once the g-2 out DMA completed
                        vector.wait_ge(dma_oe, 16 * (g - 1))
                    vector.wait_ge(pe2_sem, g + 1)
                    vector.tensor_copy(
                        o_sb[:, (g % 2) * TCMAX: (g % 2) * TCMAX + tc],
                        pt2[g % 2][:, :tc],
                    ).then_inc(dve_sem, 1)

    return nc


def _pack_inputs(xt, W1, b1, W2, idx_e, counts, cap, chunks):
    in_maps = []
    for e in range(E):
        xe = np.zeros((cap, D), dtype=np.float32)
        xe[: counts[e]] = xt[idx_e[e]]
        xeT = xe.T.astype(BF16)  # [D, cap]
        # chunk-major packing: per chunk a [128, KD*tc] block, contiguous
        xT = np.concatenate(
            [
                xeT[:, t0: t0 + tc].reshape(KD, 128, tc).transpose(1, 0, 2).reshape(128, KD * tc)
                for c, t0, tc in chunks
            ],
            axis=1,
        )
        xT = np.ascontiguousarray(xT)
        # w1 flat layout matches w1_sb: [p, m*KD*128 + k*128 + j] = W1[k*128+p, m*128+j]
        w1r = np.ascontiguousarray(
            W1[e].reshape(KD, 128, MH, 128).transpose(1, 2, 0, 3).reshape(128, MH * KD * 128)
        ).astype(BF16)
        # w2 flat layout matches w2_sb: [p, mo*MH*128 + k*128 + j] = W2[k*128+p, mo*128+j]
        w2r = np.ascontiguousarray(
            W2[e].reshape(MH, 128, MD, 128).transpose(1, 2, 0, 3).reshape(128, MD * MH * 128)
        ).astype(BF16)
        b1r = np.ascontiguousarray(b1[e].reshape(MH, 128).T)
        in_maps.append({"xT": xT, "w1": w1r, "b1t": b1r, "w2": w2r})
    return in_maps


def kernel(x, Wg, bg, W1, b1, W2, b2):
    x = np.asarray(x)
    xt = x.reshape(-1, D).astype(np.float32, copy=False)
    N = xt.shape[0]

    # --- gating on host, float64 to keep top-k selection faithful to the
    # fp32 reference (true gate margins >> fp32 rounding noise)
    logits = xt.astype(np.float64) @ np.asarray(Wg).astype(np.float64)
    logits += np.asarray(bg).astype(np.float64)
    logits -= logits.max(axis=-1, keepdims=True)
    gates = np.exp(logits)
    gates /= gates.sum(axis=-1, keepdims=True)
    order = np.argsort(-gates, axis=-1)[:, :TOP_K]            # [N, K]
    topw = np.take_along_axis(gates, order, axis=-1)          # [N, K]

    # --- per-expert token lists
    idx_e = []
    gate_e = []
    for e in range(E):
        sel = (order == e)
        rows = np.nonzero(sel.any(axis=1))[0]
        w = (topw * sel).sum(axis=1)[rows]
        idx_e.append(rows)
        gate_e.append(w.astype(np.float32))
    counts = np.array([len(r) for r in idx_e])
    cap = max(512, int(-(-counts.max() // 4) * 4))
    chunks = _chunks_of(cap)

    W1 = np.asarray(W1, dtype=np.float32)
    W2 = np.asarray(W2, dtype=np.float32)
    b1 = np.asarray(b1, dtype=np.float32)
    b2 = np.asarray(b2, dtype=np.float32)
    in_maps = _pack_inputs(xt, W1, b1, W2, idx_e, counts, cap, chunks)

    nc = _build_program(cap)

    def run_and_combine():
        res = run_bass_kernel_spmd(nc, in_maps, core_ids=list(range(E)))
        global _last_results
        _last_results = res
        out = np.zeros((N, D), dtype=np.float32)
        for e in range(E):
            ye = res.results[e]["outT"].reshape(D, cap).T  # [cap, D]
            out[idx_e[e]] += gate_e[e][:, None] * (ye[: counts[e]] + b2[e])
        return out

    def looks_wrong(out):
        if not np.isfinite(out).all():
            return True
        # spot-check a few tokens against a host fp32 recompute
        for n in (0, N // 2, N - 1):
            acc = np.zeros(D, dtype=np.float32)
            for e in order[n]:
                h = np.maximum(xt[n] @ W1[e] + b1[e], 0.0)
                acc += gates[n, e].astype(np.float32) * (h @ W2[e] + b2[e])
            if not np.allclose(out[n], acc, atol=0.05 * max(1.0, np.abs(acc).max())):
                return True
        return False

    out = run_and_combine()
    if looks_wrong(out):
        out = run_and_combine()  # one retry on transient corruption

    return out.reshape(x.shape).astype(np.float32)


# revision 33
# speedup vs baseline: 1.0698x; 1.0698x over previous
"""MoE top-2 routing kernel for Trainium2 (8 NeuronCores, expert-parallel).

Host: gating softmax + top-2 (float64 numpy), per-expert token gather,
weight re-layout + bf16 cast. Device (per core, SPMD): one expert's MLP
   h = relu(x @ W1 + b1); y = h @ W2
over that expert's routed tokens, bf16 matmuls with fp32 PSUM accumulation.
Host: combine y * gate (+ b2 folded in) via scatter-add.

Device layout: tokens in the matmul free dim, feature dims in the partition
dim, so the two layers chain without transposes and b1 is a per-partition
activation bias. Output is [D, cap] per core, transposed back on host.

DMA notes (measured): issue costs ~600ns of engine time per dma_start, a
queue ramps from ~60 to ~400GB/s over ~30us, the gpsimd queue is slow
(~40GB/s), and non-sequential DRAM reads halve throughput. So all host
arrays are laid out exactly as their SBUF destination (every DMA is a 2D
contiguous copy), the PE-start critical path (x chunk 0) is split across
the two fast queues (sync + scalar), w1 streams on sync in fine-grained
groups so layer 1 of chunk 0 is never starved, and only the non-critical
later x chunks ride the slow gpsimd queue.
"""

import numpy as np
import ml_dtypes

import concourse.bass as bass
from concourse import mybir
from concourse.bass_utils import run_bass_kernel_spmd

D = 1024
HID = 4096
E = 8
TOP_K = 2
KD = D // 128     # 8  k-blocks for layer 1
MH = HID // 128   # 32 m-blocks for layer 1 / k-blocks for layer 2
MD = D // 128     # 8  m-blocks for layer 2
TCMAX = 512       # max matmul free dim (one fp32 PSUM bank)

BF16 = ml_dtypes.bfloat16

# Weights stream over BOTH fast DMA queues (sync + scalar) in need order:
# x chunk 0 halves, then w1 in 1MB groups alternating between the queues,
# then w2 in 2MB groups alternating. Groups on the two queues use equal row
# sizes so the per-packet round-robin arbiter splits bandwidth evenly.
W1_GROUPS = [(m, m + 4) for m in range(0, MH, 4)]   # 8 groups of 4 blocks
W2_GROUPS = [(m, m + 2) for m in range(0, MD, 2)]   # 4 groups of 2 blocks

# First chunk sized so layer-1 compute of chunk 0 finishes about when w1 has
# fully streamed in (~28-40us of DMA) and w2's first blocks have landed.
TC0 = 352


def _chunks_of(cap: int):
    """Token chunks: small first chunk (rides the weight stream), remainder
    split evenly into <=512 multiple-of-4 pieces."""
    out = []
    first = min(TC0, cap)
    out.append((0, 0, first))
    t0 = first
    rest = cap - t0
    if rest > 0:
        n = -(-rest // TCMAX)
        base = -(-rest // (4 * n)) * 4
        for _ in range(n):
            tc = min(base, cap - t0)
            out.append((len(out), t0, tc))
            t0 += tc
    return out


def _build_program(cap: int):
    chunks = _chunks_of(cap)

    nc = bass.Bass()

    # all DRAM tensors are laid out exactly as their SBUF destinations
    xTd = nc.dram_tensor("xT", [128, KD * cap], mybir.dt.bfloat16, kind="ExternalInput")
    w1d = nc.dram_tensor("w1", [128, MH * KD * 128], mybir.dt.bfloat16, kind="ExternalInput")
    b1d = nc.dram_tensor("b1t", [128, MH], mybir.dt.float32, kind="ExternalInput")
    w2d = nc.dram_tensor("w2", [128, MD * MH * 128], mybir.dt.bfloat16, kind="ExternalInput")
    outd = nc.dram_tensor("outT", [MD, 128, cap], mybir.dt.float32, kind="ExternalOutput")

    from contextlib import ExitStack

    with ExitStack() as ctx:
        w1_sb = ctx.enter_context(nc.sbuf_tensor("w1_sb", [128, MH * KD * 128], mybir.dt.bfloat16))
        w2_sb = ctx.enter_context(nc.sbuf_tensor("w2_sb", [128, MD * MH * 128], mybir.dt.bfloat16))
        x_sb = ctx.enter_context(nc.sbuf_tensor("x_sb", [128, KD * cap], mybir.dt.bfloat16))
        h_sb = ctx.enter_context(nc.sbuf_tensor("h_sb", [128, MH * TCMAX], mybir.dt.bfloat16))
        o_sb = ctx.enter_context(nc.sbuf_tensor("o_sb", [128, 2 * TCMAX], mybir.dt.float32))
        b1_sb = ctx.enter_context(nc.sbuf_tensor("b1_sb", [128, MH], mybir.dt.float32))
        pt1a = ctx.enter_context(nc.psum_tensor("pt1a", [128, TCMAX], mybir.dt.float32))
        pt1b = ctx.enter_context(nc.psum_tensor("pt1b", [128, TCMAX], mybir.dt.float32))
        pt2a = ctx.enter_context(nc.psum_tensor("pt2a", [128, TCMAX], mybir.dt.float32))
        pt2b = ctx.enter_context(nc.psum_tensor("pt2b", [128, TCMAX], mybir.dt.float32))
        dma_misc = ctx.enter_context(nc.semaphore("dma_misc"))  # b1 load
        dma_xs = ctx.enter_context(nc.semaphore("dma_xs"))      # x chunk-0 half, sync
        dma_xa = ctx.enter_context(nc.semaphore("dma_xa"))      # x chunk-0 half, scalar
        dma_xg = ctx.enter_context(nc.semaphore("dma_xg"))      # +16 per later x chunk, gpsimd
        dma_w1s = ctx.enter_context(nc.semaphore("dma_w1s"))    # +16 per w1 group, sync
        dma_w1a = ctx.enter_context(nc.semaphore("dma_w1a"))    # +16 per w1 group, scalar
        dma_w2s = ctx.enter_context(nc.semaphore("dma_w2s"))    # +16 per w2 group, sync
        dma_w2a = ctx.enter_context(nc.semaphore("dma_w2a"))    # +16 per w2 group, scalar
        dma_oe = ctx.enter_context(nc.semaphore("dma_oe"))      # +16 per out DMA (sync)
        pe1_sem = ctx.enter_context(nc.semaphore("pe1_sem"))    # +1 per finished L1 m-group
        pe2_sem = ctx.enter_context(nc.semaphore("pe2_sem"))    # +1 per finished L2 mo-group
        act1_sem = ctx.enter_context(nc.semaphore("act1_sem"))  # +1 per L1 psum evict (relu)
        dve_sem = ctx.enter_context(nc.semaphore("dve_sem"))    # +1 per L2 psum evict (copy)
        block = ctx.enter_context(nc.Block())

        pt1 = [pt1a, pt1b]
        pt2 = [pt2a, pt2b]
        # chunk-major x column offsets (shared by DRAM and SBUF layouts)
        xoff = []
        off = 0
        for c, t0, tc in chunks:
            xoff.append(off)
            off += KD * tc
        tc0 = chunks[0][2]
        xhalf = KD // 2 * tc0  # split point of chunk 0's columns

        w1_sem = [dma_w1s, dma_w1a]
        w2_sem = [dma_w2s, dma_w2a]

        def issue_weight_stream(eng, parity, w1g_sem, w2g_sem):
            for i, (m0, m1) in enumerate(W1_GROUPS):
                if i % 2 != parity:
                    continue
                eng.dma_start(
                    out=w1_sb[:, m0 * KD * 128: m1 * KD * 128],
                    in_=w1d[:, m0 * KD * 128: m1 * KD * 128],
                ).then_inc(w1g_sem, 16)
            for i, (m0, m1) in enumerate(W2_GROUPS):
                if i % 2 != parity:
                    continue
                eng.dma_start(
                    out=w2_sb[:, m0 * MH * 128: m1 * MH * 128],
                    in_=w2d[:, m0 * MH * 128: m1 * MH * 128],
                ).then_inc(w2g_sem, 16)

        @block.sync
        def _(sync):
            sync.dma_start(
                out=x_sb[:, 0: xhalf], in_=xTd[:, 0: xhalf],
            ).then_inc(dma_xs, 16)
            issue_weight_stream(sync, 0, dma_w1s, dma_w2s)
            for c, t0, tc in chunks:
                for mo in range(MD):
                    g = c * MD + mo
                    sync.wait_ge(dve_sem, g + 1)
                    sync.dma_start(
                        out=outd[mo, :, t0: t0 + tc],
                        in_=o_sb[:, (g % 2) * TCMAX: (g % 2) * TCMAX + tc],
                    ).then_inc(dma_oe, 16)

        @block.scalar
        def _(scalar):
            scalar.dma_start(
                out=x_sb[:, xhalf: KD * tc0], in_=xTd[:, xhalf: KD * tc0],
            ).then_inc(dma_xa, 16)
            scalar.dma_start(out=b1_sb[:], in_=b1d[:]).then_inc(dma_misc, 16)
            issue_weight_stream(scalar, 1, dma_w1a, dma_w2a)
            scalar.wait_ge(dma_misc, 16)
            for c, t0, tc in chunks:
                for m in range(MH):
                    if m == 0 and c > 0:
                        # h_sb reused: wait until L2 of chunk c-1 consumed it
                        scalar.wait_ge(pe2_sem, MD * c)
                    scalar.wait_ge(pe1_sem, c * MH + m + 1)
                    scalar.activation(
                        h_sb[:, m * TCMAX: m * TCMAX + tc],
                        pt1[m % 2][:, :tc],
                        mybir.ActivationFunctionType.Relu,
                        bias=b1_sb[:, m: m + 1],
                    ).then_inc(act1_sem, 1)

        @block.gpsimd
        def _(gpsimd):
            for c, t0, tc in chunks[1:]:
                gpsimd.dma_start(
                    out=x_sb[:, xoff[c]: xoff[c] + KD * tc],
                    in_=xTd[:, xoff[c]: xoff[c] + KD * tc],
                ).then_inc(dma_xg, 16)

        @block.tensor
        def _(tensor):
            tensor.wait_ge(dma_xs, 16)
            tensor.wait_ge(dma_xa, 16)
            for c, t0, tc in chunks:
                if c > 0:
                    tensor.wait_ge(dma_xg, 16 * c)
                # layer 1: h[m*128+p, t] = relu(sum_d W1[d, m*128+p] x[d, t] + b1)
                for m in range(MH):
                    if c == 0 and m % 4 == 0:
                        gi = m // 4
                        tensor.wait_ge(w1_sem[gi % 2], 16 * (gi // 2 + 1))
                    g1 = c * MH + m
                    if g1 >= 2:
                        tensor.wait_ge(act1_sem, g1 - 1)  # psum bank m%2 evicted
                    ps = pt1[m % 2]
                    for k in range(KD):
                        mm = tensor.matmul(
                            ps[:, :tc],
                            w1_sb[:, (m * KD + k) * 128: (m * KD + k + 1) * 128],
                            x_sb[:, xoff[c] + k * tc: xoff[c] + (k + 1) * tc],
                            start=(k == 0),
                            stop=(k == KD - 1),
                        )
                    mm.then_inc(pe1_sem, 1)
                # layer 2: y[mo*128+p, t] = sum_hid W2[hid, mo*128+p] h[hid, t]
                tensor.wait_ge(act1_sem, MH * (c + 1))  # all h of this chunk ready
                for mo in range(MD):
                    if c == 0 and mo % 2 == 0:
                        gj = mo // 2
                        tensor.wait_ge(w2_sem[gj % 2], 16 * (gj // 2 + 1))
                    g2 = c * MD + mo
                    if g2 >= 2:
                        tensor.wait_ge(dve_sem, g2 - 1)  # psum bank mo%2 evicted
                    ps = pt2[mo % 2]
                    for k in range(MH):
                        mm = tensor.matmul(
                            ps[:, :tc],
                            w2_sb[:, (mo * MH + k) * 128: (mo * MH + k + 1) * 128],
                            h_sb[:, k * TCMAX: k * TCMAX + tc],
                            start=(k == 0),
                            stop=(k == MH - 1),
                        )
                    mm.then_inc(pe2_sem, 1)

        @block.vector
        def _(vector):
            for c, t0, tc in chunks:
                for mo in range(MD):
                    g = c * MD + mo
                    if g >= 2:
                        # o_sb slot g%2 free once the g-2 out DMA completed
                        vector.wait_ge(dma_oe, 16 * (g - 1))
                    vector.wait_ge(pe2_sem, g + 1)
                    vector.tensor_copy(
                        o_sb[:, (g % 2) * TCMAX: (g % 2) * TCMAX + tc],
                        pt2[g % 2][:, :tc],
                    ).then_inc(dve_sem, 1)

    return nc


def _pack_inputs(xt, W1, b1, W2, idx_e, counts, cap, chunks):
    in_maps = []
    for e in range(E):
        xe = np.zeros((cap, D), dtype=np.float32)
        xe[: counts[e]] = xt[idx_e[e]]
        xeT = xe.T.astype(BF16)  # [D, cap]
        # chunk-major packing: per chunk a [128, KD*tc] block, contiguous
        xT = np.concatenate(
            [
                xeT[:, t0: t0 + tc].reshape(KD, 128, tc).transpose(1, 0, 2).reshape(128, KD * tc)
                for c, t0, tc in chunks
            ],
            axis=1,
        )
        xT = np.ascontiguousarray(xT)
        # w1 flat layout matches w1_sb: [p, m*KD*128 + k*128 + j] = W1[k*128+p, m*128+j]
        w1r = np.ascontiguousarray(
            W1[e].reshape(KD, 128, MH, 128).transpose(1, 2, 0, 3).reshape(128, MH * KD * 128)
        ).astype(BF16)
        # w2 flat layout matches w2_sb: [p, mo*MH*128 + k*128 + j] = W2[k*128+p, mo*128+j]
        w2r = np.ascontiguousarray(
            W2[e].reshape(MH, 128, MD, 128).transpose(1, 2, 0, 3).reshape(128, MD * MH * 128)
        ).astype(BF16)
        b1r = np.ascontiguousarray(b1[e].reshape(MH, 128).T)
        in_maps.append({"xT": xT, "w1": w1r, "b1t": b1r, "w2": w2r})
    return in_maps


def kernel(x, Wg, bg, W1, b1, W2, b2):
    x = np.asarray(x)
    xt = x.reshape(-1, D).astype(np.float32, copy=False)
    N = xt.shape[0]

    # --- gating on host, float64 to keep top-k selection faithful to the
    # fp32 reference (true gate margins >> fp32 rounding noise)
    logits = xt.astype(np.float64) @ np.asarray(Wg).astype(np.float64)
    logits += np.asarray(bg).astype(np.float64)
    logits -= logits.max(axis=-1, keepdims=True)
    gates = np.exp(logits)
    gates /= gates.sum(axis=-1, keepdims=True)
    order = np.argsort(-gates, axis=-1)[:, :TOP_K]            # [N, K]
    topw = np.take_along_axis(gates, order, axis=-1)          # [N, K]

    # --- per-expert token lists
    idx_e = []
    gate_e = []
    for e in range(E):
        sel = (order == e)
        rows = np.nonzero(sel.any(axis=1))[0]
        w = (topw * sel).sum(axis=1)[rows]
        idx_e.append(rows)
        gate_e.append(w.astype(np.float32))
    counts = np.array([len(r) for r in idx_e])
    cap = max(512, int(-(-counts.max() // 4) * 4))
    chunks = _chunks_of(cap)

    W1 = np.asarray(W1, dtype=np.float32)
    W2 = np.asarray(W2, dtype=np.float32)
    b1 = np.asarray(b1, dtype=np.float32)
    b2 = np.asarray(b2, dtype=np.float32)
    in_maps = _pack_inputs(xt, W1, b1, W2, idx_e, counts, cap, chunks)

    nc = _build_program(cap)

    def run_and_combine():
        res = run_bass_kernel_spmd(nc, in_maps, core_ids=list(range(E)))
        global _last_results
        _last_results = res
        out = np.zeros((N, D), dtype=np.float32)
        for e in range(E):
            ye = res.results[e]["outT"].reshape(D, cap).T  # [cap, D]
            out[idx_e[e]] += gate_e[e][:, None] * (ye[: counts[e]] + b2[e])
        return out

    def looks_wrong(out):
        if not np.isfinite(out).all():
            return True
        # spot-check a few tokens against a host fp32 recompute
        for n in (0, N // 2, N - 1):
            acc = np.zeros(D, dtype=np.float32)
            for e in order[n]:
                h = np.maximum(xt[n] @ W1[e] + b1[e], 0.0)
                acc += gates[n, e].astype(np.float32) * (h @ W2[e] + b2[e])
            if not np.allclose(out[n], acc, atol=0.05 * max(1.0, np.abs(acc).max())):
                return True
        return False

    out = run_and_combine()
    if looks_wrong(out):
        out = run_and_combine()  # one retry on transient corruption

    return out.reshape(x.shape).astype(np.float32)
